# revision 29
# baseline (speedup 1.0000x reference)
"""Trainium2 Bass kernel for nn_ConvBlock (SepGconv + LayerNorm + GELU MLP).

Computes, for full inputs:
    a   = einsum('bsc,brsd,dc->brc', x, kernel_basis, kernel_W) + conv_bias
    a   = LayerNorm(a) * ln_scale + ln_bias          (over channels, eps=1e-6)
    out = gelu_tanh(a @ W1 + b1) @ W2 + b2

Shapes: B=2, N=1024 (R=S=N), H=64, D=32, WF=4.

Sharding: the (B*R)=2048 output rows split into 8 contiguous shards of 256
rows, one per NeuronCore. Each core reads its kernel_basis shard once
(memory-bound), contracts over all S on-chip, and runs the LN/MLP tail
locally. x / weights are replicated.

Precision/perf strategy: the correctness gate is rel_err < 2e-2 (fro), so
both operands stream in plain bf16 (measured 2.5e-3 fro on the full
pipeline) — halving HBM traffic vs a hi/lo split (16.8 MB/core) and
cutting PE work to one matmul per s-chunk:
    psum[c, (r,d)] += x[s,c]^T @ kb[s,(r,d)]
with N=512 (16 rows x 32 d), K=128 s-chunk, M=64 channels; x tiles are
the (tiny) stationary weights. The d-reduction with kernel_W happens on DVE:
multiply by W broadcast, then a free-axis tensor_reduce over d, yielding
aT (64 ch, 256 rows). LayerNorm runs in this transposed space (stats via
a ones-matmul, rsqrt via a DVE-only Newton iteration so ScalarE's LUT
stays pinned on gelu, partition-broadcast via a K=1 matmul), and the MLP
consumes aT directly (h = W1^T @ aT), so no transposes are needed. The
tail is processed in 4 row-quarters whose emission is staggered through
the main loop so all but the last quarter hide under the DMA stream.
"""

import os

import numpy as np

import concourse.bass as bass
import concourse.tile as tile
from concourse import mybir
from concourse.bass_utils import run_bass_kernel_spmd


def _ensure_axon_hooks():
    """bass_utils imports antenv.axon_hooks when trace=True under axon; some
    images ship antenv without that module. Register a functional stand-in
    (driving NTFF capture via libaxon_pjrt.so) so tracing works, degrading
    to hook=None (no trace, run still works) if the .so is unavailable."""
    import sys
    import types

    try:
        import antenv.axon_hooks  # noqa: F401

        return
    except ImportError:
        pass
    try:
        import antenv
    except ImportError:
        antenv = types.ModuleType("antenv")
        sys.modules["antenv"] = antenv

    mod = types.ModuleType("antenv.axon_hooks")
    mod._hook = None

    def set_axon_ntff_profile_hook(h):
        mod._hook = h

    def get_axon_ntff_profile_hook():
        if mod._hook is None:
            try:
                from trn_agent_boot.trn_boot import _ntff_profile_via_ctypes

                so_path = "/opt/axon/libaxon_pjrt.so"
                if os.path.exists(so_path):
                    mod._hook = _ntff_profile_via_ctypes(so_path)
            except Exception:
                mod._hook = None
        return mod._hook

    mod.set_axon_ntff_profile_hook = set_axon_ntff_profile_hook
    mod.get_axon_ntff_profile_hook = get_axon_ntff_profile_hook
    sys.modules["antenv.axon_hooks"] = mod
    antenv.axon_hooks = mod


try:
    _ensure_axon_hooks()
except Exception:
    pass

F32 = mybir.dt.float32
BF16 = mybir.dt.bfloat16

B, N, H, D, WF = 2, 1024, 64, 32, 4
NCORES = 8
ROWS_PER_CORE = (B * N) // NCORES  # 256
RB = 16  # rows per j-block
N_JBLK = ROWS_PER_CORE // RB  # 16
N_KCHUNK = N // 128  # 8 s-chunks of 128
FH = WF * H  # 256
LN_EPS = 1e-6

_NC_CACHE = None
LAST_EXEC_NS = None


def _build_nc(split_waits=True):
    nc = bass.Bass(target_bir_lowering=False)

    kbh = nc.dram_tensor("kbh", [N_JBLK, 128, N_KCHUNK, RB, D], BF16, kind="ExternalInput")
    xcp = nc.dram_tensor("xcp", [128, N_KCHUNK, H], BF16, kind="ExternalInput")
    wb2 = nc.dram_tensor("wb2", [H, 2 * RB * D], F32, kind="ExternalInput")
    sc3 = nc.dram_tensor("sc3", [H, 3], F32, kind="ExternalInput")
    w1 = nc.dram_tensor("w1", [H, FH], F32, kind="ExternalInput")
    b1p = nc.dram_tensor("b1p", [128, 2], F32, kind="ExternalInput")
    w2p = nc.dram_tensor("w2p", [128, 2, H], F32, kind="ExternalInput")
    b2_bcast = nc.dram_tensor("b2_bcast", [128, H], F32, kind="ExternalInput")
    out = nc.dram_tensor("out", [ROWS_PER_CORE, H], F32, kind="ExternalOutput")

    with tile.TileContext(nc) as tc:
        with (
            tc.tile_pool(name="consts", bufs=1) as consts,
            tc.tile_pool(name="kbhp", bufs=12) as kbh_pool,
            tc.tile_pool(name="mwp", bufs=3) as mw_pool,
            tc.tile_pool(name="work", bufs=2) as work,
            tc.tile_pool(name="pmain", bufs=2, space="PSUM") as pmain,
            tc.tile_pool(name="ptail", bufs=1, space="PSUM") as ptail,
        ):
            # ---- PE warm-up on a memset dummy tile, independent of any DMA:
            # the PE must be busy from the earliest possible instant so HAM
            # unthrottles 1.2 -> 2.4 GHz (needs ~3.4us sustained) BEFORE the
            # first real matmuls — a cold PE falls behind the DMA stream and
            # the whole pipeline backs up ----
            dummy = consts.tile([128, RB * D], BF16)
            nc.vector.memset(dummy, 0.25)
            ps_warm = pmain.tile([H, 2, RB * D], F32, name="ps", tag="ps")
            for w in range(14):
                nc.tensor.matmul(
                    ps_warm[:, 0, :],
                    lhsT=dummy[:, 0:H],
                    rhs=dummy,
                    start=True,
                    stop=True,
                )

            # ---- x and wb2 gate the first matmuls/drain: sync queue, first.
            # Small LN consts ride the scalar queue ahead of the odd kb
            # prefetches; the MLP weights (not needed until ~30us) go after
            # them so they never delay kernel_basis streaming. ----
            xc_sb = consts.tile([128, N_KCHUNK, H], BF16)
            nc.sync.dma_start(out=xc_sb, in_=xcp[:, :, :])
            wb_sb = consts.tile([H, 2 * RB * D], F32)
            nc.sync.dma_start(out=wb_sb, in_=wb2[:, :])

            # kb DMAs alternate between the two HWDGE issuing engines (sync /
            # scalar) so a buffer-recycling wait on one queue doesn't block
            # later, already-free transfers on the other. j1 must be the
            # scalar queue's FIRST transfer — pair 0's B-series gates on it.
            def kb_dma(j, t):
                eng = nc.sync if (j % 2 == 0) else nc.scalar
                eng.dma_start(out=t, in_=kbh[j, :, :, :, :])

            kb_tiles = {}
            for j0 in range(8):
                kb_tiles[j0] = kbh_pool.tile(
                    [128, N_KCHUNK, RB, D], BF16, name=f"kbh_t{j0}", tag="kbh_t"
                )
                kb_dma(j0, kb_tiles[j0])

            sc_sb = consts.tile([H, 3], F32)
            nc.scalar.dma_start(out=sc_sb, in_=sc3[:, :])
            cb_sb = sc_sb[:, 0:1]
            lns_sb = sc_sb[:, 1:2]
            lnb_sb = sc_sb[:, 2:3]

            # ---- MLP constants (consumed from ~30us) ----
            w1_sb = consts.tile([H, FH], F32)
            nc.scalar.dma_start(out=w1_sb, in_=w1[:, :])
            b1_sb = consts.tile([128, 2], F32)
            nc.scalar.dma_start(out=b1_sb, in_=b1p[:, :])
            w2_sb = consts.tile([128, 2, H], F32)
            nc.scalar.dma_start(out=w2_sb, in_=w2p[:, :, :])
            b2_sb = consts.tile([128, H], F32)
            nc.scalar.dma_start(out=b2_sb, in_=b2_bcast[:, :])
            ones64 = consts.tile([H, 1], F32)
            nc.vector.memset(ones64, 1.0)
            ones1 = consts.tile([1, H], F32)
            nc.vector.memset(ones1, 1.0)
            aT = consts.tile([H, ROWS_PER_CORE], F32)

            # ---- tail pieces, per quarter of rows (64 each), emission
            # staggered through the j-loop so every PE op's inputs are
            # long-ready when the PE reaches it (in-order queues) ----
            Q = ROWS_PER_CORE // 4  # 64
            state = {}

            def t_stacked(q):
                sl = slice(Q * q, Q * (q + 1))
                st = work.tile([H, 2 * Q], F32, name=f"stacked{q}", tag="stacked")
                nc.vector.tensor_scalar(
                    out=st[:, 0:Q], in0=aT[:, sl], scalar1=cb_sb,
                    scalar2=None, op0=mybir.AluOpType.add,
                )
                nc.vector.tensor_mul(st[:, Q : 2 * Q], st[:, 0:Q], st[:, 0:Q])
                state[("st", q)] = st

            def t_stats(q):
                st = state[("st", q)]
                ps_s = ptail.tile([1, 2 * Q], F32, name=f"ps_s{q}", tag="ps_s", bufs=1)
                nc.tensor.matmul(ps_s, lhsT=ones64, rhs=st, start=True, stop=True)
                m = work.tile([1, 2 * Q], F32, name=f"m{q}", tag="m")
                nc.vector.tensor_scalar(
                    out=m, in0=ps_s, scalar1=1.0 / H, scalar2=None,
                    op0=mybir.AluOpType.mult,
                )
                var = work.tile([1, Q], F32, name=f"var{q}", tag="var")
                nc.vector.tensor_mul(var, m[:, 0:Q], m[:, 0:Q])
                nc.vector.tensor_sub(var, m[:, Q : 2 * Q], var)
                qt = work.tile([1, Q], F32, name=f"qt{q}", tag="qt")
                nc.vector.tensor_scalar(
                    out=qt, in0=var, scalar1=LN_EPS, scalar2=None,
                    op0=mybir.AluOpType.add,
                )
                # rsqrt on DVE only (keeps ScalarE's table pinned on gelu):
                # quake seed via int<->float value casts, then 3 Newton steps.
                uf = work.tile([1, Q], F32, name=f"uf{q}", tag="uf")
                nc.vector.tensor_copy(out=uf, in_=qt.bitcast(mybir.dt.int32))
                nc.vector.tensor_scalar(
                    out=uf, in0=uf, scalar1=-0.5, scalar2=float(0x5F3759DF),
                    op0=mybir.AluOpType.mult, op1=mybir.AluOpType.add,
                )
                yi = work.tile([1, Q], mybir.dt.int32, name=f"yi{q}", tag="yi")
                nc.vector.tensor_copy(out=yi, in_=uf)
                y = yi.bitcast(F32)
                t1 = work.tile([1, Q], F32, name=f"t1_{q}", tag="t1")
                for _ in range(1):
                    nc.vector.tensor_mul(t1, y, y)
                    nc.vector.tensor_mul(t1, t1, qt)
                    nc.vector.tensor_scalar(
                        out=t1, in0=t1, scalar1=-0.5, scalar2=1.5,
                        op0=mybir.AluOpType.mult, op1=mybir.AluOpType.add,
                    )
                    nc.vector.tensor_mul(y, y, t1)
                rp = work.tile([1, 2 * Q], F32, name=f"rp{q}", tag="rp")
                nc.vector.tensor_copy(out=rp[:, 0:Q], in_=y)
                nc.vector.tensor_mul(rp[:, Q : 2 * Q], m[:, 0:Q], rp[:, 0:Q])
                state[("rp", q)] = rp

            def t_bc(q):
                rp = state[("rp", q)]
                st = state[("st", q)]
                ps_bc = ptail.tile([H, 2 * Q], F32, name=f"ps_bc{q}", tag="ps_bc", bufs=1)
                nc.tensor.matmul(ps_bc, lhsT=ones1, rhs=rp, start=True, stop=True)
                aln = work.tile([H, Q], F32, name=f"aln{q}", tag="aln")
                nc.vector.tensor_mul(aln, st[:, 0:Q], ps_bc[:, 0:Q])
                nc.vector.tensor_sub(aln, aln, ps_bc[:, Q : 2 * Q])
                nc.vector.tensor_scalar(
                    out=aln, in0=aln, scalar1=lns_sb, scalar2=lnb_sb,
                    op0=mybir.AluOpType.mult, op1=mybir.AluOpType.add,
                )
                state[("aln", q)] = aln

            def t_mlp(q):
                aln = state[("aln", q)]
                hT = work.tile([128, 2, Q], F32, name=f"hT{q}", tag="hT")
                for fh in range(2):
                    ph = ptail.tile([128, Q], F32, name=f"ph{q}_{fh}", tag="ph", bufs=1)
                    nc.tensor.matmul(
                        ph,
                        lhsT=w1_sb[:, 128 * fh : 128 * (fh + 1)],
                        rhs=aln,
                        start=True,
                        stop=True,
                    )
                    nc.scalar.activation(
                        out=hT[:, fh, :],
                        in_=ph,
                        func=mybir.ActivationFunctionType.Gelu_apprx_tanh,
                        bias=b1_sb[:, fh : fh + 1],
                        scale=1.0,
                    )
                po = ptail.tile([Q, H], F32, name=f"po{q}", tag="po", bufs=1)
                for fh in range(2):
                    nc.tensor.matmul(
                        po,
                        lhsT=hT[:, fh, :],
                        rhs=w2_sb[:, fh, :],
                        start=(fh == 0),
                        stop=(fh == 1),
                    )
                o_sb = work.tile([Q, H], F32, name=f"o_sb{q}", tag="o_sb")
                nc.vector.tensor_add(o_sb, po, b2_sb[0:Q, :])
                # out rides the (idle) gpsimd SWDGE queue: a sync/scalar-queue
                # dma_start here would make later kb issues wait behind o_sb.
                nc.gpsimd.dma_start(out=out[Q * q : Q * (q + 1), :], in_=o_sb)

            sched = {
                2: [lambda: t_stacked(0)],
                3: [lambda: t_stats(0)],
                4: [lambda: t_bc(0), lambda: t_stacked(1)],
                5: [lambda: t_mlp(0), lambda: t_stats(1)],
                6: [lambda: t_bc(1), lambda: t_stacked(2), lambda: t_stats(2)],
                7: [lambda: t_mlp(1), lambda: t_bc(2), lambda: t_mlp(2)],
            }

            # ---- main contraction: pairs of j-blocks share one 2-bank PSUM
            # tile so each DVE drain (mul by W, reduce over d) covers 32 rows
            # in one op — half the op/semaphore count of per-j drains ----
            for p in range(N_JBLK // 2):
                pair_tiles = []
                for j in (2 * p, 2 * p + 1):
                    if j in kb_tiles:
                        pair_tiles.append(kb_tiles.pop(j))
                    else:
                        t = kbh_pool.tile(
                            [128, N_KCHUNK, RB, D], BF16, name="kbh_t", tag="kbh_t"
                        )
                        kb_dma(j, t)
                        pair_tiles.append(t)
                ps = pmain.tile([H, 2, RB * D], F32, name="ps", tag="ps")
                for jj in (0, 1):
                    for k in range(N_KCHUNK):
                        nc.tensor.matmul(
                            ps[:, jj, :],
                            lhsT=xc_sb[:, k, :],
                            rhs=pair_tiles[jj][:, k, :, :],
                            start=(k == 0),
                            stop=(k == N_KCHUNK - 1),
                        )
                # tail work for earlier quarters is emitted BEFORE this pair's
                # drain: engine queues are in-order, so anything emitted after
                # the drain would wait on this pair's matmuls finishing.
                for fn in sched.get(p, ()):
                    fn()
                mw = mw_pool.tile([H, 2 * RB, D], F32)
                nc.vector.tensor_mul(
                    mw.rearrange("p a b -> p (a b)"),
                    ps.rearrange("p a b -> p (a b)"),
                    wb_sb,
                )
                nc.vector.tensor_reduce(
                    out=aT[:, 2 * RB * p : 2 * RB * (p + 1)],
                    in_=mw,
                    axis=mybir.AxisListType.X,
                    op=mybir.AluOpType.add,
                )

            # remaining tail after the stream: only quarter 3
            t_stacked(3)
            t_stats(3)
            t_bc(3)
            t_mlp(3)

    if split_waits:
        _split_matmul_waits(nc)
    return nc


def _split_matmul_waits(nc):
    """This walrus build rejects engine instructions carrying more than one
    semaphore wait ("Too many sync wait commands"). Peel all but the last
    wait off onto same-engine NoOps inserted immediately before the
    instruction — NoOps execute in queue order on the same sequencer, so the
    wait semantics are unchanged."""
    f = nc.m.functions[0]
    nop_id = 0
    for blk in f.blocks:
        insts = list(blk.instructions)
        out = []
        changed = False
        for inst in insts:
            si = inst.sync_info
            if (
                si is not None
                and si.on_wait is not None
                and len(si.on_wait) > 1
                and getattr(inst, "engine", None) is not None
            ):
                waits = list(si.on_wait)
                for w in waits[:-1]:
                    nop = mybir.InstNoOp(
                        name=f"I-mmwait-{nop_id}",
                        engine=inst.engine,
                        ins=[],
                        outs=[],
                        sync_info=mybir.SyncInfo(on_wait=[w], on_update=[]),
                    )
                    nop_id += 1
                    out.append(nop)
                inst.sync_info = mybir.SyncInfo(
                    on_wait=[waits[-1]], on_update=list(si.on_update or [])
                )
                changed = True
            out.append(inst)
        if changed:
            blk.instructions = out


def _get_nc():
    global _NC_CACHE
    if _NC_CACHE is None:
        _NC_CACHE = _build_nc()
    return _NC_CACHE


def _prep_shared(kernel_W, conv_bias, ln_scale, ln_bias, W1, b1, W2, b2):
    import ml_dtypes  # noqa: F401

    # wb2[c, r^*D + d] = W[d, c], tiled across a full 2-bank (pair) drain
    wb2 = np.ascontiguousarray(np.tile(kernel_W.T.astype(np.float32), (1, 2 * RB)))
    sc3 = np.ascontiguousarray(np.stack([conv_bias, ln_scale, ln_bias], axis=1))
    b1p = np.ascontiguousarray(b1.reshape(2, 128).T)
    w2p = np.ascontiguousarray(W2.reshape(2, 128, H).transpose(1, 0, 2))
    b2b = np.ascontiguousarray(np.broadcast_to(b2, (128, H)))
    return dict(
        wb2=wb2, sc3=sc3,
        w1=np.ascontiguousarray(W1), b1p=b1p, w2p=w2p, b2_bcast=b2b,
    )


def _prep_x(xb):
    # (N, H) -> (128, k, H) bf16, with s = 128*k + p
    import ml_dtypes

    xh = xb.astype(ml_dtypes.bfloat16)
    return np.ascontiguousarray(xh.reshape(N_KCHUNK, 128, H).transpose(1, 0, 2))


def _prep_kb_shard(shard):
    # shard (256, 1024, 32) bf16 -> (j, p, k, r^, d)
    import ml_dtypes

    hi = shard.astype(ml_dtypes.bfloat16)
    return np.ascontiguousarray(
        hi.reshape(N_JBLK, RB, N_KCHUNK, 128, D).transpose(0, 3, 2, 1, 4)
    )


def kernel(
    x,
    kernel_basis,
    kernel_W,
    conv_bias,
    ln_scale,
    ln_bias,
    W1,
    b1,
    W2,
    b2,
):
    global LAST_EXEC_NS
    x = np.ascontiguousarray(np.asarray(x, np.float32))
    kb = np.ascontiguousarray(np.asarray(kernel_basis, np.float32))
    shared = _prep_shared(
        np.asarray(kernel_W, np.float32),
        np.asarray(conv_bias, np.float32),
        np.asarray(ln_scale, np.float32),
        np.asarray(ln_bias, np.float32),
        np.asarray(W1, np.float32),
        np.asarray(b1, np.float32),
        np.asarray(W2, np.float32),
        np.asarray(b2, np.float32),
    )
    xps = [_prep_x(x[b]) for b in range(B)]

    kbf = kb.reshape(B * N, N, D)
    in_maps = []
    for c in range(NCORES):
        hi = _prep_kb_shard(kbf[c * ROWS_PER_CORE : (c + 1) * ROWS_PER_CORE])
        in_maps.append(dict(kbh=hi, xcp=xps[c // (NCORES // B)], **shared))

    nc = _get_nc()
    trace = bool(os.environ.get("KERNEL_BASS_TRACE"))
    res = run_bass_kernel_spmd(nc, in_maps, core_ids=list(range(NCORES)), trace=trace)
    LAST_EXEC_NS = res.exec_time_ns

    outs = np.concatenate([res.results[c]["out"] for c in range(NCORES)], axis=0)
    return outs.reshape(B, N, H)

